# revision 2
# baseline (speedup 1.0000x reference)
"""COIL sparse-attention scoring kernel for 8 Trainium2 NeuronCores.

Band-chunk strategy
-------------------
Shard the doc axis (Bd=128) across the 8 cores (16 docs each); qry tensors are
replicated.  The exact-token-match mask is folded INTO the matmul: each token
id (vocab 1000) is encoded as three base-10 digit one-hots scaled by ALPHA=32
and appended to the reps, so v[qs, ct] = S[qs, ct] + 1024 * match_digits and
relu(v - 3072) isolates exact matches (non-match v stays below ~2100).

Unlike the dense-cartesian formulation, query rows and doc tokens are
CO-PARTITIONED BY TOKEN ID on the host: all query rows and all slab tokens of
a given id land in the same 128x128 chunk (8 queries x 16 row slots vs 128
token slots).  Matches can only occur inside a chunk, so per core the scored
area shrinks from 3712x2048 to NCH*128*128 (~16x less PSUM to reduce).  Each
chunk batch is relu'd (ScalarE/DVE alternating) to bf16 and collapsed over the
fixed slot->query map with a single [128,8] selector matmul; the [8, cols]
result streams straight from PSUM to HBM.

The reference's masked max over doc tokens equals this relu-sum except when a
(query row, doc) pair has >=2 matching tokens with positive scores; those rare
collision sets (~0.7% of pairs) are corrected exactly on the host during
assembly, where the per-doc fold, CLS scores and the final max over the 8
query chunks already run.
"""

import os
import numpy as np
import ml_dtypes

Bq, Sq, Bd, Sd, D, Dc = 8, 512, 128, 128, 32, 768
NCORES = 8
BD_PER = Bd // NCORES          # 16 docs per core
TOK = BD_PER * Sd              # 2048 slab tokens per core
SQF = Bq * Sq                  # 4096 query positions
K_EXT = 126                    # 32*3 hi/lo rep dims + 30 one-hot dims
ALPHA = 32.0
OFF = 3.0 * ALPHA * ALPHA      # 3072: offset of a full 3-digit match
SLOTS_PER_Q = 16               # fixed row slots per query per chunk
BATCH = int(os.environ.get("KERNEL_BATCH", "4"))       # chunks per relu batch
WARMUP_MMS = int(os.environ.get("KERNEL_WARMUP_MMS", "12"))

_CACHE = {}


def _bf16(x):
    return x.astype(ml_dtypes.bfloat16)


def _onehot_digits(ids):
    """ids [N] int in [0,1000) -> [N,30] base-10 digit one-hot (float32)."""
    n = ids.shape[0]
    H = np.zeros((n, 30), dtype=np.float32)
    r = np.arange(n)
    H[r, ids % 10] = 1.0
    H[r, 10 + (ids // 10) % 10] = 1.0
    H[r, 20 + ids // 100] = 1.0
    return H


def _ext_rows(reps, ids, order):
    """hi/lo bf16 split + digit one-hots -> [N, 126] f32 rows."""
    x = np.asarray(reps, np.float32).reshape(-1, D)
    hi = _bf16(x).astype(np.float32)
    lo = _bf16(x - hi).astype(np.float32)
    H = ALPHA * _onehot_digits(ids)
    if order == "q":
        return np.concatenate([hi, lo, hi, H], axis=1)
    return np.concatenate([hi, hi, lo, H], axis=1)


def _qry_row_mask(inputs):
    """[Bq, Sq] bool: rows that can contribute (attended, not CLS/SEP)."""
    mask = np.asarray(inputs["qry_attention_mask"], np.int64).copy()
    sep = mask.sum(axis=1) - 1
    mask[np.arange(Bq), sep] = 0
    mask[:, 0] = 0
    return mask.astype(bool)


def _pack_core(qids, qrow_q, rows, dids):
    """Greedy co-partition of query rows + slab tokens by id.

    Returns list of chunks: (rows_by_q: list of 8 lists of row idx,
    toks: list of token idx).  Each chunk holds <=SLOTS_PER_Q rows per query
    and <=128 tokens, with every id's rows+tokens fully inside one chunk.
    """
    packs = []
    for v in np.unique(dids):
        rv = rows[qids[rows] == v]
        tv = np.nonzero(dids == v)[0]
        byq = [rv[qrow_q[rv] == qq] for qq in range(Bq)]
        packs.append((max(len(b) for b in byq), byq, tv))
    packs.sort(key=lambda p: -p[0])
    chunks = []
    for _, byq, tv in packs:
        need_q = [len(b) for b in byq]
        nt = len(tv)
        for ch in chunks:
            if ch["nt"] + nt <= 128 and all(
                ch["fill"][qq] + need_q[qq] <= SLOTS_PER_Q for qq in range(Bq)
            ):
                break
        else:
            ch = {"fill": [0] * Bq, "rows": [[] for _ in range(Bq)],
                  "toks": [], "nt": 0}
            chunks.append(ch)
        for qq in range(Bq):
            ch["rows"][qq].extend(byq[qq].tolist())
            ch["fill"][qq] += need_q[qq]
        ch["toks"].extend(tv.tolist())
        ch["nt"] += nt
    return chunks


def _split_multi_waits(nc, mybir):
    """This container's walrus accepts only ONE sync-wait per instruction
    ("Too many sync wait commands"). Hoist extra waits into standalone
    EventSemaphore instructions on the same engine right before the offender
    (the sequencer blocks on each in order — semantically identical)."""
    n = 0
    for func in nc.m.functions:
        for bb in func.blocks:
            out = []
            for inst in bb.instructions:
                si = inst.sync_info
                if si is not None and len(si.on_wait) > 1:
                    waits = list(si.on_wait)
                    for w in waits[:-1]:
                        n += 1
                        out.append(
                            mybir.InstEventSemaphore(
                                name=f"W-{inst.name}-{n}",
                                engine=inst.engine,
                                ins=[],
                                outs=[],
                                debug=inst.debug,
                                sync_info=mybir.SyncInfo(
                                    on_wait=[w], on_update=[]
                                ),
                            )
                        )
                    inst.sync_info = mybir.SyncInfo(
                        on_wait=[waits[-1]], on_update=list(si.on_update)
                    )
                out.append(inst)
            bb.instructions = out
    return n


def _build_nc(nch):
    import concourse.bass as bass
    import concourse.mybir as mybir
    import concourse.tile as tile

    bf16, f32 = mybir.dt.bfloat16, mybir.dt.float32
    nc = bass.Bass("TRN2", target_bir_lowering=False, debug=False)
    # [126, x] transfers spread across all 16 SDMA engines (<=64-partition
    # transfers only reach half of them via the partition->port swizzle)
    cols = nch * 128
    qryP = nc.dram_tensor("qryP", [K_EXT, cols], bf16, kind="ExternalInput").ap()
    docP = nc.dram_tensor("docP", [K_EXT, cols], bf16, kind="ExternalInput").ap()
    selT = nc.dram_tensor("selT", [128, 8], bf16, kind="ExternalInput").ap()
    out = nc.dram_tensor("out", [128, 8 * nch], f32, kind="ExternalOutput").ap()

    if os.environ.get("KERNEL_MINI"):
        # overhead floor measurement: same I/O contract, trivial compute
        with tile.TileContext(nc) as tc:
            with tc.tile_pool(name="p", bufs=1) as pool:
                t = pool.tile([128, 8], f32)
                sel_sb = pool.tile([128, 8], bf16)
                nc.sync.dma_start(sel_sb[:], selT[:])
                acc = pool.tile([128, 8 * nch], f32)
                nc.vector.memset(acc[:], 0.0)
                nc.sync.dma_start(out[:], acc[:])
        _split_multi_waits(nc, mybir)
        return nc

    batches = []
    c = 0
    while c < nch:
        batches.append((c, min(BATCH, nch - c)))
        c += BATCH

    with tile.TileContext(nc) as tc:
        with (
            tc.tile_pool(name="inp", bufs=1) as inp,
            tc.tile_pool(name="psum", bufs=3, space="PSUM") as psum,
            tc.tile_pool(name="sout", bufs=2, space="PSUM") as sop,
            tc.tile_pool(name="stage", bufs=3) as stp,
        ):
            # PE warm-up: back-to-back junk matmuls during the DMA head so
            # the HAM clock-gate (3.4us sustained-busy window) flips to full
            # rate before the real work; alternating PSUM tiles avoid WAW
            # serialization so the busy window is contiguous
            scratch = inp.tile([128, 512], bf16)
            nc.vector.memset(scratch[:], 0.0)
            for _ in range(WARMUP_MMS):
                wps = psum.tile([128, 512], f32, tag="score")
                nc.tensor.matmul(
                    wps[:], scratch[:, 0:128], scratch[:], start=True, stop=True
                )

            qry_sb = inp.tile([K_EXT, cols], bf16)
            doc_sb = inp.tile([K_EXT, cols], bf16)
            sel_sb = inp.tile([128, 8], bf16)
            negoff = inp.tile([128, 1], f32)
            nc.vector.memset(negoff[:], -OFF)
            # dummy activation: pull the Relu table load into the DMA head
            # instead of blocking the first real ScalarE relu
            warm_act = stp.tile([128, 1], bf16, tag="warm")
            nc.scalar.activation(
                warm_act[:], negoff[:],
                mybir.ActivationFunctionType.Relu, bias=negoff[:],
            )
            nc.sync.dma_start(sel_sb[:], selT[:])
            # batch-major 512-col transfers on the two parallel HWDGE rings
            # (sync=SP, scalar=ACT) so batch b's operands land early together
            for off in range(0, cols, 512):
                w = min(512, cols - off)
                nc.sync.dma_start(qry_sb[:, off : off + w], qryP[:, off : off + w])
                nc.scalar.dma_start(doc_sb[:, off : off + w], docP[:, off : off + w])

            accum = inp.tile([128, 8 * nch], f32)
            # all selector outputs land in ONE persistent PSUM bank (8*nch
            # fp32 cols <= 512); per batch a [128, 8*bn] slab is copied out
            # and DMA'd immediately so the final HBM-write completion ack
            # (~2us+) overlaps compute instead of serializing at the end
            so = sop.tile([128, 8 * nch], f32)
            for bi, (c0, bn) in enumerate(batches):
                ps = psum.tile([128, bn * 128], f32, tag="score")
                for j in range(bn):
                    lo = (c0 + j) * 128
                    nc.tensor.matmul(
                        ps[:, j * 128 : (j + 1) * 128],
                        qry_sb[:, lo : lo + 128],
                        doc_sb[:, lo : lo + 128],
                        start=True,
                        stop=True,
                    )
                srel = stp.tile([128, bn * 128], bf16, tag="stage")
                if bi % 2 == 0:
                    # relu(v - OFF) on DVE via max/add tensor_scalar
                    nc.vector.tensor_scalar(
                        srel[:], ps[:], OFF, -OFF,
                        mybir.AluOpType.max, mybir.AluOpType.add,
                    )
                else:
                    nc.scalar.activation(
                        srel[:], ps[:],
                        mybir.ActivationFunctionType.Relu,
                        bias=negoff[:],
                    )
                for j in range(bn):
                    # fold the 16 slots per query: stationary = relu'd chunk
                    # [128qs, 128tok], moving = slot->query map -> [128tok, 8]
                    c = c0 + j
                    nc.tensor.matmul(
                        so[:, c * 8 : c * 8 + 8],
                        srel[:, j * 128 : (j + 1) * 128],
                        sel_sb[:],
                        start=True, stop=True,
                    )
            nc.vector.tensor_copy(accum[:], so[:])
            nc.sync.dma_start(out[:], accum[:])
    _split_multi_waits(nc, mybir)
    return nc


def _get_nc(nch):
    key = (nch, BATCH)
    if key not in _CACHE:
        _CACHE[key] = _build_nc(nch)
    return _CACHE[key]


def _prepare(inputs):
    """Host prep: id-banded packing per core -> qryP/docP/selT + fold maps."""
    qids = np.asarray(inputs["qry_input_ids"], np.int64).reshape(SQF)
    qext = _ext_rows(inputs["qry_reps"], qids, "q")          # [SQF, 126]
    row_ok = _qry_row_mask(inputs).reshape(SQF)
    qrow_q = np.repeat(np.arange(Bq), Sq)
    dids_all = np.asarray(inputs["doc_input_ids"], np.int64).reshape(-1)
    dext_all = _ext_rows(inputs["doc_reps"], dids_all, "d")  # [Bd*Sd, 126]

    per_core = []
    for core in range(NCORES):
        tsl = slice(core * TOK, (core + 1) * TOK)
        dids = dids_all[tsl]
        rows = np.nonzero(row_ok & np.isin(qids, np.unique(dids)))[0]
        chunks = _pack_core(qids, qrow_q, rows, dids)
        per_core.append((chunks, tsl))
    nch = max(len(c) for c, _ in per_core)

    in_maps, folds = [], []
    for core in range(NCORES):
        chunks, tsl = per_core[core]
        dext, dids = dext_all[tsl], dids_all[tsl]
        cols = nch * 128
        qP = np.zeros((cols, K_EXT), np.float32)
        dP = np.zeros((cols, K_EXT), np.float32)
        col_doc = np.full(cols, -1, np.int64)   # (chunk,slot) -> doc
        for n, ch in enumerate(chunks):
            for qq in range(Bq):
                base = n * 128 + qq * SLOTS_PER_Q
                r = ch["rows"][qq]
                if r:
                    qP[base : base + len(r)] = qext[r]
            t = np.asarray(ch["toks"], np.int64)
            dP[n * 128 : n * 128 + len(t)] = dext[t]
            col_doc[n * 128 : n * 128 + len(t)] = t // Sd
        sel = np.zeros((128, 8), np.float32)
        sel[np.arange(128), np.arange(128) // SLOTS_PER_Q] = 1.0
        in_maps.append({
            "qryP": np.ascontiguousarray(_bf16(qP.T)),
            "docP": np.ascontiguousarray(_bf16(dP.T)),
            "selT": _bf16(sel),
        })
        folds.append(col_doc)
    return in_maps, folds, nch


def _corrections(inputs):
    """[Bq, Bd] exact fix for (row, doc) pairs with >=2 matching tokens:
    the device sums relu'd match scores; the reference takes their max."""
    qids = np.asarray(inputs["qry_input_ids"], np.int64).reshape(SQF)
    qreps = np.asarray(inputs["qry_reps"], np.float32).reshape(SQF, D)
    dids = np.asarray(inputs["doc_input_ids"], np.int64)    # [Bd, Sd]
    dreps = np.asarray(inputs["doc_reps"], np.float32)      # [Bd, Sd, D]
    row_ok = _qry_row_mask(inputs).reshape(SQF)
    qrow_q = np.repeat(np.arange(Bq), Sq)
    corr = np.zeros((Bq, Bd), np.float32)
    for d in range(Bd):
        ids_d = dids[d]
        vals, cnt = np.unique(ids_d, return_counts=True)
        for v in vals[cnt >= 2]:
            rv = np.nonzero(row_ok & (qids == v))[0]
            if len(rv) == 0:
                continue
            S = qreps[rv] @ dreps[d, ids_d == v].T
            R = np.maximum(S, 0.0)
            delta = R.max(axis=1) - R.sum(axis=1)       # <= 0
            np.add.at(corr[:, d], qrow_q[rv], delta)
    return corr


def _assemble(inputs, results, folds, nch):
    toks = np.zeros((Bq, Bd), dtype=np.float32)
    for core in range(NCORES):
        o = np.asarray(results[core]["out"], np.float32)    # [128, 8*nch]
        # [tok_slot, chunk, q] -> [chunk*128+tok_slot, q]
        arr = o.reshape(128, nch, 8).transpose(1, 0, 2).reshape(nch * 128, 8)
        col_doc = folds[core]
        ok = col_doc >= 0
        part = np.zeros((Bq, BD_PER), np.float32)
        np.add.at(part.T, col_doc[ok], arr[ok])
        toks[:, core * BD_PER : (core + 1) * BD_PER] = part
    toks += _corrections(inputs)
    cls = np.asarray(inputs["qry_cls"], np.float32) @ np.asarray(
        inputs["doc_cls"], np.float32
    ).T
    return (toks + cls).max(axis=0).reshape(-1).astype(np.float32)


def _ensure_ntff_hook():
    """This container's antenv lacks axon_hooks; synthesize the module and
    register the ctypes-based NTFF profile hook so trace=True works."""
    import sys
    import types

    if "antenv.axon_hooks" in sys.modules:
        return
    mod = types.ModuleType("antenv.axon_hooks")
    state = {"hook": None}
    mod.set_axon_ntff_profile_hook = lambda h: state.__setitem__("hook", h)
    mod.get_axon_ntff_profile_hook = lambda: state["hook"]
    sys.modules["antenv.axon_hooks"] = mod
    try:
        import antenv

        antenv.axon_hooks = mod
    except ImportError:
        pass
    try:
        from trn_agent_boot.trn_boot import _ntff_profile_via_ctypes

        mod.set_axon_ntff_profile_hook(
            _ntff_profile_via_ctypes("/opt/axon/libaxon_pjrt.so")
        )
    except Exception:
        pass


def run(inputs, trace=False, **kwargs):
    """Run on the 8 NeuronCores; returns (output, BassKernelResults)."""
    from concourse.bass_utils import run_bass_kernel_spmd

    if trace:
        _ensure_ntff_hook()
    in_maps, folds, nch = _prepare(inputs)
    nc = _get_nc(nch)
    res = run_bass_kernel_spmd(
        nc, in_maps, core_ids=list(range(NCORES)), trace=trace, **kwargs
    )
    return _assemble(inputs, res.results, folds, nch), res


def kernel(**inputs) -> np.ndarray:
    out, _ = run(inputs)
    return out


# revision 3
# speedup vs baseline: 1.0223x; 1.0223x over previous
"""COIL sparse-attention scoring kernel for 8 Trainium2 NeuronCores.

Band-chunk strategy
-------------------
Shard the doc axis (Bd=128) across the 8 cores (16 docs each); qry tensors are
replicated.  The exact-token-match mask is folded INTO the matmul: each token
id (vocab 1000) is encoded as three base-10 digit one-hots scaled by ALPHA=32
and appended to the reps, so v[qs, ct] = S[qs, ct] + 1024 * match_digits and
relu(v - 3072) isolates exact matches (non-match v stays below ~2100).

Unlike the dense-cartesian formulation, query rows and doc tokens are
CO-PARTITIONED BY TOKEN ID on the host: all query rows and all slab tokens of
a given id land in the same 128x128 chunk (8 queries x 16 row slots vs 128
token slots).  Matches can only occur inside a chunk, so per core the scored
area shrinks from 3712x2048 to NCH*128*128 (~16x less PSUM to reduce).  Each
chunk batch is relu'd (ScalarE/DVE alternating) to bf16 and collapsed over the
fixed slot->query map with a single [128,8] selector matmul; the [8, cols]
result streams straight from PSUM to HBM.

The reference's masked max over doc tokens equals this relu-sum except when a
(query row, doc) pair has >=2 matching tokens with positive scores; those rare
collision sets (~0.7% of pairs) are corrected exactly on the host during
assembly, where the per-doc fold, CLS scores and the final max over the 8
query chunks already run.
"""

import os
import numpy as np
import ml_dtypes

Bq, Sq, Bd, Sd, D, Dc = 8, 512, 128, 128, 32, 768
NCORES = 8
BD_PER = Bd // NCORES          # 16 docs per core
TOK = BD_PER * Sd              # 2048 slab tokens per core
SQF = Bq * Sq                  # 4096 query positions
K_EXT = 126                    # 32*3 hi/lo rep dims + 30 one-hot dims
ALPHA = 32.0
OFF = 3.0 * ALPHA * ALPHA      # 3072: offset of a full 3-digit match
SLOTS_PER_Q = 16               # fixed row slots per query per chunk
BATCH = int(os.environ.get("KERNEL_BATCH", "4"))       # chunks per relu batch
WARMUP_MMS = int(os.environ.get("KERNEL_WARMUP_MMS", "9"))

_CACHE = {}


def _bf16(x):
    return x.astype(ml_dtypes.bfloat16)


def _onehot_digits(ids):
    """ids [N] int in [0,1000) -> [N,30] base-10 digit one-hot (float32)."""
    n = ids.shape[0]
    H = np.zeros((n, 30), dtype=np.float32)
    r = np.arange(n)
    H[r, ids % 10] = 1.0
    H[r, 10 + (ids // 10) % 10] = 1.0
    H[r, 20 + ids // 100] = 1.0
    return H


def _ext_rows(reps, ids, order):
    """hi/lo bf16 split + digit one-hots -> [N, 126] f32 rows."""
    x = np.asarray(reps, np.float32).reshape(-1, D)
    hi = _bf16(x).astype(np.float32)
    lo = _bf16(x - hi).astype(np.float32)
    H = ALPHA * _onehot_digits(ids)
    if order == "q":
        return np.concatenate([hi, lo, hi, H], axis=1)
    return np.concatenate([hi, hi, lo, H], axis=1)


def _qry_row_mask(inputs):
    """[Bq, Sq] bool: rows that can contribute (attended, not CLS/SEP)."""
    mask = np.asarray(inputs["qry_attention_mask"], np.int64).copy()
    sep = mask.sum(axis=1) - 1
    mask[np.arange(Bq), sep] = 0
    mask[:, 0] = 0
    return mask.astype(bool)


def _pack_core(qids, qrow_q, rows, dids):
    """Greedy co-partition of query rows + slab tokens by id.

    Returns list of chunks: (rows_by_q: list of 8 lists of row idx,
    toks: list of token idx).  Each chunk holds <=SLOTS_PER_Q rows per query
    and <=128 tokens, with every id's rows+tokens fully inside one chunk.
    """
    packs = []
    for v in np.unique(dids):
        rv = rows[qids[rows] == v]
        tv = np.nonzero(dids == v)[0]
        byq = [rv[qrow_q[rv] == qq] for qq in range(Bq)]
        packs.append((max(len(b) for b in byq), byq, tv))
    packs.sort(key=lambda p: -p[0])
    chunks = []
    for _, byq, tv in packs:
        need_q = [len(b) for b in byq]
        nt = len(tv)
        for ch in chunks:
            if ch["nt"] + nt <= 128 and all(
                ch["fill"][qq] + need_q[qq] <= SLOTS_PER_Q for qq in range(Bq)
            ):
                break
        else:
            ch = {"fill": [0] * Bq, "rows": [[] for _ in range(Bq)],
                  "toks": [], "nt": 0}
            chunks.append(ch)
        for qq in range(Bq):
            ch["rows"][qq].extend(byq[qq].tolist())
            ch["fill"][qq] += need_q[qq]
        ch["toks"].extend(tv.tolist())
        ch["nt"] += nt
    return chunks


def _split_multi_waits(nc, mybir):
    """This container's walrus accepts only ONE sync-wait per instruction
    ("Too many sync wait commands"). Hoist extra waits into standalone
    EventSemaphore instructions on the same engine right before the offender
    (the sequencer blocks on each in order — semantically identical)."""
    n = 0
    for func in nc.m.functions:
        for bb in func.blocks:
            out = []
            for inst in bb.instructions:
                si = inst.sync_info
                if si is not None and len(si.on_wait) > 1:
                    waits = list(si.on_wait)
                    for w in waits[:-1]:
                        n += 1
                        out.append(
                            mybir.InstEventSemaphore(
                                name=f"W-{inst.name}-{n}",
                                engine=inst.engine,
                                ins=[],
                                outs=[],
                                debug=inst.debug,
                                sync_info=mybir.SyncInfo(
                                    on_wait=[w], on_update=[]
                                ),
                            )
                        )
                    inst.sync_info = mybir.SyncInfo(
                        on_wait=[waits[-1]], on_update=list(si.on_update)
                    )
                out.append(inst)
            bb.instructions = out
    return n


def _build_nc(nch):
    import concourse.bass as bass
    import concourse.mybir as mybir
    import concourse.tile as tile

    bf16, f32 = mybir.dt.bfloat16, mybir.dt.float32
    nc = bass.Bass("TRN2", target_bir_lowering=False, debug=False)
    # [126, x] transfers spread across all 16 SDMA engines (<=64-partition
    # transfers only reach half of them via the partition->port swizzle)
    cols = nch * 128
    qryP = nc.dram_tensor("qryP", [K_EXT, cols], bf16, kind="ExternalInput").ap()
    docP = nc.dram_tensor("docP", [K_EXT, cols], bf16, kind="ExternalInput").ap()
    selT = nc.dram_tensor("selT", [128, 8], bf16, kind="ExternalInput").ap()
    out = nc.dram_tensor("out", [128, 8 * nch], f32, kind="ExternalOutput").ap()

    if os.environ.get("KERNEL_MINI"):
        # overhead floor measurement: same I/O contract, trivial compute
        with tile.TileContext(nc) as tc:
            with tc.tile_pool(name="p", bufs=1) as pool:
                t = pool.tile([128, 8], f32)
                sel_sb = pool.tile([128, 8], bf16)
                nc.sync.dma_start(sel_sb[:], selT[:])
                acc = pool.tile([128, 8 * nch], f32)
                nc.vector.memset(acc[:], 0.0)
                nc.sync.dma_start(out[:], acc[:])
        _split_multi_waits(nc, mybir)
        return nc

    batches = []
    c = 0
    while c < nch:
        batches.append((c, min(BATCH, nch - c)))
        c += BATCH

    with tile.TileContext(nc) as tc:
        with (
            tc.tile_pool(name="inp", bufs=1) as inp,
            tc.tile_pool(name="psum", bufs=3, space="PSUM") as psum,
            tc.tile_pool(name="sout", bufs=2, space="PSUM") as sop,
            tc.tile_pool(name="stage", bufs=3) as stp,
        ):
            # PE warm-up: back-to-back junk matmuls during the DMA head so
            # the HAM clock-gate (3.4us sustained-busy window) flips to full
            # rate before the real work; alternating PSUM tiles avoid WAW
            # serialization so the busy window is contiguous
            scratch = inp.tile([128, 512], bf16)
            nc.gpsimd.memset(scratch[:], 0.0)
            for _ in range(WARMUP_MMS):
                wps = psum.tile([128, 512], f32, tag="score")
                nc.tensor.matmul(
                    wps[:], scratch[:, 0:128], scratch[:], start=True, stop=True
                )

            qry_sb = inp.tile([K_EXT, cols], bf16)
            doc_sb = inp.tile([K_EXT, cols], bf16)
            sel_sb = inp.tile([128, 8], bf16)
            negoff = inp.tile([128, 1], f32)
            nc.vector.memset(negoff[:], -OFF)
            # dummy activation: pull the Relu table load into the DMA head
            # instead of blocking the first real ScalarE relu
            warm_act = stp.tile([128, 1], bf16, tag="warm")
            nc.scalar.activation(
                warm_act[:], negoff[:],
                mybir.ActivationFunctionType.Relu, bias=negoff[:],
            )
            nc.sync.dma_start(sel_sb[:], selT[:])
            # batch-major 512-col transfers on the two parallel HWDGE rings
            # (sync=SP, scalar=ACT) so batch b's operands land early together
            for off in range(0, cols, 512):
                w = min(512, cols - off)
                nc.sync.dma_start(qry_sb[:, off : off + w], qryP[:, off : off + w])
                nc.scalar.dma_start(doc_sb[:, off : off + w], docP[:, off : off + w])

            accum = inp.tile([128, 8 * nch], f32)
            # all selector outputs land in ONE persistent PSUM bank (8*nch
            # fp32 cols <= 512); per batch a [128, 8*bn] slab is copied out
            # and DMA'd immediately so the final HBM-write completion ack
            # (~2us+) overlaps compute instead of serializing at the end
            so = sop.tile([128, 8 * nch], f32)
            for bi, (c0, bn) in enumerate(batches):
                ps = psum.tile([128, bn * 128], f32, tag="score")
                for j in range(bn):
                    lo = (c0 + j) * 128
                    nc.tensor.matmul(
                        ps[:, j * 128 : (j + 1) * 128],
                        qry_sb[:, lo : lo + 128],
                        doc_sb[:, lo : lo + 128],
                        start=True,
                        stop=True,
                    )
                srel = stp.tile([128, bn * 128], bf16, tag="stage")
                if bi % 2 == 0:
                    # relu(v - OFF) on DVE via max/add tensor_scalar
                    nc.vector.tensor_scalar(
                        srel[:], ps[:], OFF, -OFF,
                        mybir.AluOpType.max, mybir.AluOpType.add,
                    )
                else:
                    nc.scalar.activation(
                        srel[:], ps[:],
                        mybir.ActivationFunctionType.Relu,
                        bias=negoff[:],
                    )
                for j in range(bn):
                    # fold the 16 slots per query: stationary = relu'd chunk
                    # [128qs, 128tok], moving = slot->query map -> [128tok, 8]
                    c = c0 + j
                    nc.tensor.matmul(
                        so[:, c * 8 : c * 8 + 8],
                        srel[:, j * 128 : (j + 1) * 128],
                        sel_sb[:],
                        start=True, stop=True,
                    )
            nc.vector.tensor_copy(accum[:], so[:])
            nc.sync.dma_start(out[:], accum[:])
    _split_multi_waits(nc, mybir)
    return nc


def _get_nc(nch):
    key = (nch, BATCH)
    if key not in _CACHE:
        _CACHE[key] = _build_nc(nch)
    return _CACHE[key]


def _prepare(inputs):
    """Host prep: id-banded packing per core -> qryP/docP/selT + fold maps."""
    qids = np.asarray(inputs["qry_input_ids"], np.int64).reshape(SQF)
    qext = _ext_rows(inputs["qry_reps"], qids, "q")          # [SQF, 126]
    row_ok = _qry_row_mask(inputs).reshape(SQF)
    qrow_q = np.repeat(np.arange(Bq), Sq)
    dids_all = np.asarray(inputs["doc_input_ids"], np.int64).reshape(-1)
    dext_all = _ext_rows(inputs["doc_reps"], dids_all, "d")  # [Bd*Sd, 126]

    per_core = []
    for core in range(NCORES):
        tsl = slice(core * TOK, (core + 1) * TOK)
        dids = dids_all[tsl]
        rows = np.nonzero(row_ok & np.isin(qids, np.unique(dids)))[0]
        chunks = _pack_core(qids, qrow_q, rows, dids)
        per_core.append((chunks, tsl))
    nch = max(len(c) for c, _ in per_core)

    in_maps, folds = [], []
    for core in range(NCORES):
        chunks, tsl = per_core[core]
        dext, dids = dext_all[tsl], dids_all[tsl]
        cols = nch * 128
        qP = np.zeros((cols, K_EXT), np.float32)
        dP = np.zeros((cols, K_EXT), np.float32)
        col_doc = np.full(cols, -1, np.int64)   # (chunk,slot) -> doc
        for n, ch in enumerate(chunks):
            for qq in range(Bq):
                base = n * 128 + qq * SLOTS_PER_Q
                r = ch["rows"][qq]
                if r:
                    qP[base : base + len(r)] = qext[r]
            t = np.asarray(ch["toks"], np.int64)
            dP[n * 128 : n * 128 + len(t)] = dext[t]
            col_doc[n * 128 : n * 128 + len(t)] = t // Sd
        sel = np.zeros((128, 8), np.float32)
        sel[np.arange(128), np.arange(128) // SLOTS_PER_Q] = 1.0
        in_maps.append({
            "qryP": np.ascontiguousarray(_bf16(qP.T)),
            "docP": np.ascontiguousarray(_bf16(dP.T)),
            "selT": _bf16(sel),
        })
        folds.append(col_doc)
    return in_maps, folds, nch


def _corrections(inputs):
    """[Bq, Bd] exact fix for (row, doc) pairs with >=2 matching tokens:
    the device sums relu'd match scores; the reference takes their max."""
    qids = np.asarray(inputs["qry_input_ids"], np.int64).reshape(SQF)
    qreps = np.asarray(inputs["qry_reps"], np.float32).reshape(SQF, D)
    dids = np.asarray(inputs["doc_input_ids"], np.int64)    # [Bd, Sd]
    dreps = np.asarray(inputs["doc_reps"], np.float32)      # [Bd, Sd, D]
    row_ok = _qry_row_mask(inputs).reshape(SQF)
    qrow_q = np.repeat(np.arange(Bq), Sq)
    corr = np.zeros((Bq, Bd), np.float32)
    for d in range(Bd):
        ids_d = dids[d]
        vals, cnt = np.unique(ids_d, return_counts=True)
        for v in vals[cnt >= 2]:
            rv = np.nonzero(row_ok & (qids == v))[0]
            if len(rv) == 0:
                continue
            S = qreps[rv] @ dreps[d, ids_d == v].T
            R = np.maximum(S, 0.0)
            delta = R.max(axis=1) - R.sum(axis=1)       # <= 0
            np.add.at(corr[:, d], qrow_q[rv], delta)
    return corr


def _assemble(inputs, results, folds, nch):
    toks = np.zeros((Bq, Bd), dtype=np.float32)
    for core in range(NCORES):
        o = np.asarray(results[core]["out"], np.float32)    # [128, 8*nch]
        # [tok_slot, chunk, q] -> [chunk*128+tok_slot, q]
        arr = o.reshape(128, nch, 8).transpose(1, 0, 2).reshape(nch * 128, 8)
        col_doc = folds[core]
        ok = col_doc >= 0
        part = np.zeros((Bq, BD_PER), np.float32)
        np.add.at(part.T, col_doc[ok], arr[ok])
        toks[:, core * BD_PER : (core + 1) * BD_PER] = part
    toks += _corrections(inputs)
    cls = np.asarray(inputs["qry_cls"], np.float32) @ np.asarray(
        inputs["doc_cls"], np.float32
    ).T
    return (toks + cls).max(axis=0).reshape(-1).astype(np.float32)


def _ensure_ntff_hook():
    """This container's antenv lacks axon_hooks; synthesize the module and
    register the ctypes-based NTFF profile hook so trace=True works."""
    import sys
    import types

    if "antenv.axon_hooks" in sys.modules:
        return
    mod = types.ModuleType("antenv.axon_hooks")
    state = {"hook": None}
    mod.set_axon_ntff_profile_hook = lambda h: state.__setitem__("hook", h)
    mod.get_axon_ntff_profile_hook = lambda: state["hook"]
    sys.modules["antenv.axon_hooks"] = mod
    try:
        import antenv

        antenv.axon_hooks = mod
    except ImportError:
        pass
    try:
        from trn_agent_boot.trn_boot import _ntff_profile_via_ctypes

        mod.set_axon_ntff_profile_hook(
            _ntff_profile_via_ctypes("/opt/axon/libaxon_pjrt.so")
        )
    except Exception:
        pass


def run(inputs, trace=False, **kwargs):
    """Run on the 8 NeuronCores; returns (output, BassKernelResults)."""
    from concourse.bass_utils import run_bass_kernel_spmd

    if trace:
        _ensure_ntff_hook()
    in_maps, folds, nch = _prepare(inputs)
    nc = _get_nc(nch)
    res = run_bass_kernel_spmd(
        nc, in_maps, core_ids=list(range(NCORES)), trace=trace, **kwargs
    )
    return _assemble(inputs, res.results, folds, nch), res


def kernel(**inputs) -> np.ndarray:
    out, _ = run(inputs)
    return out
